# revision 130
# baseline (speedup 1.0000x reference)
"""BEiT-style attention (B=16, N=577, C=768, H=12) on 8 TRN2 NeuronCores.

Strategy: pure data-parallel over batch (2 batches/core, no collectives).
Per-core kernel computes attention in a transposed-score layout (S^T with
softmax axis on partitions) which needs zero on-device transposes:

  qT,kT  [d, n] = Wqk8 @ x8^T          fp8 DoubleRow, 1 pass  (weights x ws)
  vT     [m, d] = x8-slices @ Wv8      fp8 DoubleRow, 3-pass residual
  S^T    [m, n] = kT.T-slices @ qT     fp8 DoubleRow (K=64 contraction)
  expS^T [m, n] = exp(S^T) * exp(relposT)   (exp(bias) precomputed on host)
  outT   [d, n] = [1 | ws*v].T @ expS^T     (row 64 = softmax denominator)
  out    [n,co] = outT_norm.T-slices @ (Wp/ws)^T + bias   fp16

fp8 weights are pre-scaled by ws=32 on the host to stay clear of e4m3
subnormals; the inverse is folded into the psum evictions (q/k) or into the
fp16 proj weights (v path). All non-fp8 on-device tensors are fp16.
"""

import os
import sys
from collections import deque
from contextlib import ExitStack

import numpy as np

sys.path.insert(0, "/opt/trn_rl_repo")

# the kernel executes through jax/PJRT on the axon-tunneled NeuronCores; a
# JAX_PLATFORMS=cpu pin (useful for pure-reference runs) would hide them
if os.environ.get("JAX_PLATFORMS", "") == "cpu":
    os.environ.pop("JAX_PLATFORMS", None)

import ml_dtypes  # noqa: E402

from concourse import bacc, mybir  # noqa: E402
import concourse.bass as bass  # noqa: E402
import concourse.tile as tile  # noqa: E402
from concourse.bass_utils import run_bass_kernel_spmd  # noqa: E402

F16 = mybir.dt.float16
F32 = mybir.dt.float32
F8 = mybir.dt.float8e4
NPF16 = np.float16
NPF8 = ml_dtypes.float8_e4m3
AF = mybir.ActivationFunctionType
DR = mybir.MatmulPerfMode.DoubleRow
MULT = mybir.AluOpType.mult
ADD = mybir.AluOpType.add

B, N, C = 16, 577, 768
H, HD = 12, 64
NCORES = 8
BL = B // NCORES  # local batches per core
KC = C // 128  # 128-row channel blocks
KB = 3  # 256-row DoubleRow channel blocks
SCALE = HD ** -0.5
# fp8 ranges for the S^T inputs: q carries SCALE*QS8, k carries KS8; the
# product's 1/(QS8*KS8) is folded into the Exp activation's scale operand
QS8, KS8 = 4.0, 2.0
EXPSC = 1.0 / (QS8 * KS8)
# fp8 weight pre-scale (keeps the 0.02-sigma weights out of e4m3 subnormals)
WS = 32.0
QEVS = SCALE * QS8 / WS  # q psum -> q8 eviction scale
KEVS = KS8 / WS          # k psum -> k8 eviction scale
# v path: vt carries WS*v; 1/WS is folded into the fp16 proj weights

# eviction-engine assignment: which engine evicts qk / v psums
QK_EV = "dve"  # dve | act | alt
V_EV = "alt"
EST_M4 = "pool"  # dve | pool: engine for the last (single-tile) est multiply
EST_PAIR_POOL = 2  # pass-2 est pair-mults to Pool (0..2)
EST_PAIR_POOL_B0 = 2  # pass-1 est pair-mults to Pool
PV_MERGE = True  # one 2-bank pv tile (bufs=1, merged recip/norm) vs FB-split
INNER_EV = "dve"  # eviction-engine override for fills popped inside a head

# token-dim partition tiles (offset, width)
NT = [(0, 128), (128, 128), (256, 128), (384, 128), (512, 65)]
# chunks aligned to PSUM bank boundary for fused two-bank [.,577] psum tiles
FB = [(0, 512), (512, 65)]
# channel free chunks for 768-wide outputs
CC = [(0, 384), (384, 384)]


def build_graph():
    nc = bacc.Bacc("TRN2", target_bir_lowering=False, debug=False, num_devices=NCORES)

    # fp8 DoubleRow operands arrive pre-packed per partition: every [2, w]
    # block is (i, cols)-contiguous (pack stride = w) as the dual-fp8
    # Ldweights/ISA path requires. x comes in two packings: FB-chunk-blocked
    # (x8q: moving side of q/k) and m-tile-blocked (x8m/xrm: stationary side
    # of v). Column layouts (per kb block of 2N=1154):
    #   x8q: [i0 n512 | i1 n512 | i0 n65 | i1 n65]       (2N per kb)
    #   x8m: [m-tile blocks of (i0 128 | i1 128)]*5      (1280 per kb,
    #        m4 zero-padded to 128 — dual-fp8 Ldweights needs 128-wide blocks)
    x8q_d = nc.dram_tensor("x8q", (BL, 128, KB * 2 * N), F8, kind="ExternalInput").ap()
    x8m_d = nc.dram_tensor("x8m", (BL, 128, KB * 1280), F8, kind="ExternalInput").ap()
    xrm_d = nc.dram_tensor("xrm", (BL, 128, KB * 1280), F8, kind="ExternalInput").ap()
    wqk_d = nc.dram_tensor("wqk8", (128, KB * 12 * 256), F8, kind="ExternalInput").ap()
    wv_d = nc.dram_tensor("wv8", (128, KB * 2 * C), F8, kind="ExternalInput").ap()
    wvr_d = nc.dram_tensor("wvr8", (128, KB * 2 * C), F8, kind="ExternalInput").ap()
    pw_d = nc.dram_tensor("pwT", (C, C), F16, kind="ExternalInput").ap()
    # eb rows padded 577 -> 640 per head so each head loads as ONE clean
    # [p, 5, 577] DMA (the pad rows land in the ebt tile's unused tail)
    eb_d = nc.dram_tensor("ebT", (H * 640, N), F16, kind="ExternalInput").ap()
    qkb_d = nc.dram_tensor("qkb", (128, 2 * KC), F32, kind="ExternalInput").ap()
    pbc_d = nc.dram_tensor("pbc", (128, KC), F32, kind="ExternalInput").ap()
    out_d = nc.dram_tensor("out", (BL, C, N), F16, kind="ExternalOutput").ap()

    with tile.TileContext(nc) as tc, ExitStack() as ctx:
        res = ctx.enter_context(tc.tile_pool(name="res", bufs=1))
        estp = ctx.enter_context(tc.tile_pool(name="estp", bufs=2))
        rowp = ctx.enter_context(tc.tile_pool(name="rowp", bufs=4))
        ps_mm = ctx.enter_context(
            tc.tile_pool(name="ps_mm", bufs=2, space=bass.MemorySpace.PSUM)
        )
        ps_st = ctx.enter_context(
            tc.tile_pool(name="ps_st", bufs=2, space=bass.MemorySpace.PSUM)
        )
        # PV_MERGE: pv tiles are two banks ([128, 577], bufs=1) so each head's
        # denominator/numerator evict as ONE recip + ONE multiply; else two
        # single-bank tiles (bufs=2) with per-FB-chunk recip/norm
        ps_pv = ctx.enter_context(
            tc.tile_pool(
                name="ps_pv", bufs=1 if PV_MERGE else 2, space=bass.MemorySpace.PSUM
            )
        )

        # ---- resident tiles + batched input DMA (issued from idle SP) ----
        # contraction channel c = kb*256 + i*128 + p for every DR operand
        wqk = res.tile([128, KB * 12 * 256], F8, name="wqk", tag="wqk")
        wv = res.tile([128, KB * 2 * C], F8, name="wv", tag="wv")
        wvr = res.tile([128, KB * 2 * C], F8, name="wvr", tag="wvr")
        pw = res.tile([128, KC * C], F16, name="pw", tag="pw")
        x8q = [res.tile([128, KB * 2 * N], F8, name=f"x8q{b}", tag=f"x8q{b}") for b in range(BL)]
        x8m = [res.tile([128, KB * 1280], F8, name=f"x8m{b}", tag=f"x8m{b}") for b in range(BL)]
        xrm = [res.tile([128, KB * 1280], F8, name=f"xrm{b}", tag=f"xrm{b}") for b in range(BL)]
        # fp8 q/k in DoubleRow layout: cluster tile g holds heads 4g..4g+3 at
        # partitions 32j..32j+32, channel d = i*32 + p (host permutes wqk's
        # output columns). Ldweights requires contiguous, 128-wide [2, m]
        # blocks -> k8 is m-tile-blocked (m4 padded to 128); the moving side
        # is flexible -> q8 is FB-chunk-blocked ([2,512] + [2,65]).
        q8 = [
            [res.tile([128, 2 * N], F8, name=f"q8_{b}_{g}", tag=f"q8_{b}_{g}") for g in range(3)]
            for b in range(BL)
        ]
        k8 = [
            [res.tile([128, 10 * 128], F8, name=f"k8_{b}_{g}", tag=f"k8_{b}_{g}") for g in range(3)]
            for b in range(BL)
        ]
        # zero the padding of the last k block so its garbage psum rows stay finite
        for b in range(BL):
            for g in range(3):
                nc.vector.memset(k8[b][g][:, 1024 + 65 : 1152], 0.0)
                nc.vector.memset(k8[b][g][:, 1152 + 65 : 1280], 0.0)
        # per head: [64 ones-cols | 64 v-cols]. The ones block makes the PV
        # matmul emit 64 copies of the softmax denominator on partitions 0:64
        # (out partitions = lhsT free index), so the reciprocal is directly
        # partition-parallel and the gpsimd partition_broadcast hop vanishes
        # from every normalize chain.
        vt = [
            [res.tile([128, H * 2 * HD], F16, name=f"vt{b}_{m}", tag=f"vt{b}_{m}") for m in range(len(NT))]
            for b in range(BL)
        ]
        ot = [
            [res.tile([128, N], F16, name=f"ot{b}_{k}", tag=f"ot{b}_{k}") for k in range(KC)]
            for b in range(BL)
        ]
        qkb = res.tile([128, 2 * KC], F32, name="qkb_s", tag="qkb_s")
        pbc = res.tile([128, KC], F32, name="pbc_s", tag="pbc_s")
        finp = ctx.enter_context(tc.tile_pool(name="finp", bufs=4))
        # rel-pos bias exp tiles, loaded once and shared by both batches
        ebt = [res.tile([128, 5 * N], F16, name=f"eb{h}", tag=f"eb{h}") for h in range(H)]

        # startup is DMA-bound: all engines are idle at t=0, so spread the
        # input loads across four issue queues (each queue's transfers
        # serialize on its own lane). SP keeps the pass-1 critical stream
        # (x8m/wv for the first v matmuls), Pool takes the residual pass
        # operands, ACT the q/k operands, DVE the b1 tensors.
        def dma_cols(dst, src_2d, c0, c1, eng=None):
            (eng or nc.sync).dma_start(dst[:, c0:c1], src_2d[:, c0:c1])

        dma_cols(x8m[0], x8m_d[0], 0, 1280)
        dma_cols(wv, wv_d, 0, 2 * C)
        dma_cols(x8m[0], x8m_d[0], 1280, 3 * 1280)
        dma_cols(wv, wv_d, 2 * C, 6 * C)
        dma_cols(xrm[0], xrm_d[0], 0, 3 * 1280, nc.gpsimd)
        dma_cols(wvr, wvr_d, 0, 6 * C, nc.gpsimd)
        nc.scalar.dma_start(qkb[:], qkb_d[:])
        dma_cols(x8q[0], x8q_d[0], 0, 6 * N, nc.scalar)
        dma_cols(wqk, wqk_d, 0, 12 * 256, nc.scalar)
        dma_cols(wqk, wqk_d, 12 * 256, 24 * 256, nc.scalar)
        dma_cols(wqk, wqk_d, 24 * 256, 36 * 256, nc.scalar)
        dma_cols(x8m[1], x8m_d[1], 0, 3 * 1280, nc.gpsimd)
        dma_cols(x8q[1], x8q_d[1], 0, 6 * N, nc.gpsimd)
        dma_cols(xrm[1], xrm_d[1], 0, 3 * 1280, nc.gpsimd)

        # rel-pos bias stream: one DMA per head, all issued up front,
        # round-robin over the three DMA-capable queues, ordered by the
        # pass-1 head cadence so no head waits on its bias tile
        def load_eb(h, eng):
            eng.dma_start(
                ebt[h][:, 0 : 5 * N].rearrange("p (m n) -> p m n", m=5),
                eb_d[h * 640 : (h + 1) * 640, :].rearrange("(m p) n -> p m n", p=128),
            )

        # SP only: ACT/Pool-issued transfers occupy those engines' lanes, and
        # both run hot mid-pass; SP's 2.28us/head cadence stays ahead of the
        # ~3.3us/head consumption
        for h in range(H):
            load_eb(h, nc.sync)

        def dr2(tile2d, off, w):
            """[128, 2, w] DoubleRow view of a (i, cols)-contiguous block."""
            return tile2d[:, off : off + 2 * w].rearrange("p (i c) -> p i c", i=2)

        def xm_blk(xt, kb, m):
            # uniform 128-wide blocks; m4 zero-padded (psum rows 65.. are 0)
            return dr2(xt, kb * 1280 + m * 256, 128)

        def xq_blk(xt, kb, fci):
            if fci == 0:
                return dr2(xt, kb * 2 * N, 512)
            return dr2(xt, kb * 2 * N + 1024, 65)

        def wqk_blk(kb, t):
            return dr2(wqk, (kb * 12 + t) * 256, 128)

        def wv_blk(wt, kb, ci):
            return dr2(wt, kb * 2 * C + ci * 2 * CC[0][1], CC[ci][1])

        # ones column (index HD) interleaved per head in the v tiles
        for b in range(BL):
            for m in range(len(NT)):
                vints = vt[b][m][:].rearrange("p (h e) -> p h e", h=H)
                # ones blocks on gpsimd: big strided memsets, but Pool idles
                nc.gpsimd.memset(vints[:, :, 0:HD], 1.0)

        def evict_v(b, m, ci, ps, ev=None):
            c0, cw = CC[ci]
            m0, mw = NT[m]
            nh = cw // HD
            dst = vt[b][m][:mw, ci * nh * 2 * HD : (ci + 1) * nh * 2 * HD]
            dst = dst.rearrange("p (h e) -> p h e", h=nh)[:, :, HD : 2 * HD]
            src = ps[:mw, :cw].rearrange("p (h e) -> p h e", h=nh)
            # gpsimd cannot read PSUM; psum evictions are ACT/DVE-only
            ev = ev or V_EV
            if ev == "act" or (ev == "alt" and (b + m) % 2 == 0):
                nc.scalar.activation(dst, src, AF.Identity)
            else:
                nc.vector.tensor_copy(dst, src)

        # the 3 residual passes of the v projection, kb-major within a pass
        VPASS = [(0, 0), (1, 0), (0, 1)]  # (x-residual?, w-residual?)

        def v_mms(b, m, ci, ps):
            m0, mw = NT[m]
            c0, cw = CC[ci]
            first = True
            for xi, wi in VPASS:
                xt_ = (xrm if xi else x8m)[b]
                wt_ = wvr if wi else wv
                for kb in range(KB):
                    nc.tensor.matmul(
                        ps[:, :cw],
                        xm_blk(xt_, kb, m),
                        wv_blk(wt_, kb, ci),
                        start=first,
                        stop=(xi, wi) == VPASS[-1] and kb == KB - 1,
                        perf_mode=DR,
                    )
                    first = False

        def emit_qkv_v(b, m, cis=(0, 1), ev=None):
            for ci in cis:
                ps = ps_mm.tile([128, 512], F32, name=f"psv{b}_{m}_{ci}", tag="mm")
                v_mms(b, m, ci, ps)
                evict_v(b, m, ci, ps, ev=ev)

        def emit_qkv_v0_startup():
            # PE is in-order: emit the first-ever matmuls kb-major across 5
            # accumulation groups (2 ps_mm + 2 borrowed ps_st banks + 1 ps_pv)
            # so PE tracks the incoming (x8[kb], wv-cc0[kb]) DMA stream
            c0, cw = CC[0]
            grp = []
            for m in range(5):
                if m < 2:
                    grp.append(ps_mm.tile([128, 512], F32, name=f"psv0_{m}_0", tag="mm"))
                elif m < 4:
                    grp.append(ps_st.tile([128, N], F32, name=f"psv0_{m}_0", tag="st"))
                else:
                    grp.append(ps_pv.tile([128, 512], F32, name=f"psv0_{m}_0", tag="pv"))
            for pi, (xi, wi) in enumerate(VPASS):
                xt_ = (xrm if xi else x8m)[0]
                wt_ = wvr if wi else wv
                for kb in range(KB):
                    for m in range(5):
                        nc.tensor.matmul(
                            grp[m][:, :cw],
                            xm_blk(xt_, kb, m),
                            wv_blk(wt_, kb, 0),
                            start=(pi == 0 and kb == 0),
                            stop=(pi == len(VPASS) - 1 and kb == KB - 1),
                            perf_mode=DR,
                        )
            for m in range(5):
                evict_v(0, m, 0, grp[m])

        QFC = [(0, 512), (512, 65)]  # q/k matmul chunks = x8q blocks

        def qk_unit(b, t, fci, ev=None):
            """One token chunk of qk group t (0..11). Group t = cluster
            g=t//4, part=t%4 (0,1 = q channel-halves A/B; 2,3 = k halves).
            Weights carry WS; eviction applies QEVS/KEVS, adds the
            (pre-scaled) q bias and converts to fp8 in the DR block layout."""
            n0, nw = QFC[fci]
            ps = ps_mm.tile([128, 512], F32, name=f"psqk{b}_{t}_{n0}", tag="mm")
            for kb in range(KB):
                nc.tensor.matmul(
                    ps[:, :nw],
                    wqk_blk(kb, t),
                    xq_blk(x8q[b], kb, fci),
                    start=(kb == 0),
                    stop=(kb == KB - 1),
                    perf_mode=DR,
                )
            qk_evict(b, t, fci, ps, ev)

        def qk_evict(b, t, fci, ps, ev=None):
            g, part = t // 4, t % 4
            i = part % 2
            isq = part < 2
            n0, nw = QFC[fci]
            evs = QEVS if isq else KEVS
            if isq:
                dst = (
                    q8[b][g][:, i * 512 : i * 512 + 512]
                    if fci == 0
                    else q8[b][g][:, 1024 + i * 65 : 1024 + i * 65 + 65]
                )
                src = ps[:, :nw]
            else:
                kt = k8[b][g]
                if fci == 0:
                    # 512 keys = 4 k8 m-blocks; write channel-pack i's 128
                    # key-columns of each
                    base = kt[:, 0:1024].rearrange("p (m c) -> p m c", m=4)
                    dst = base[:, :, i * 128 : (i + 1) * 128]
                    src = ps[:, :nw].rearrange("p (m c) -> p m c", m=4)
                else:
                    dst = kt[:, 1024 + i * 128 : 1024 + i * 128 + 65]
                    src = ps[:, :nw]
            # psum eviction is DVE/ACT-only (gpsimd can't read PSUM)
            ev = ev or QK_EV
            if ev == "dve" or (ev == "alt" and t % 2 == 0):
                nc.vector.tensor_scalar(dst, src, evs, qkb[:, t : t + 1], MULT, ADD)
            else:
                nc.scalar.activation(dst, src, AF.Identity, bias=qkb[:, t : t + 1], scale=evs)

        def emit_qkv_qk(b, ts):
            for t in ts:
                for fci in range(2):
                    qk_unit(b, t, fci)

        def emit_qkv_qk0():
            """Startup cluster 0, ordered so S^T(h0) unblocks after the first
            four evictions (k fc0 pair -> q fc0 pair -> q tails -> k tails),
            alternating eviction engines (ACT is idle during the ramp)."""
            for n, (t, fci) in enumerate(
                [(2, 0), (3, 0), (0, 0), (1, 0), (0, 1), (1, 1), (2, 1), (3, 1)]
            ):
                qk_unit(0, t, fci, ev="act" if n % 2 == 0 else "dve")



        def emit_att_st(h, b, eb, fill=None, inner=0):
            """fp8 DoubleRow S^T + exp + bias-mult, pair-merged: the 512-wide
            main chunks of each m-pair share one [128,1024] 2-bank psum tile
            (ONE exp per pair), and all five 65-wide tail chunks batch into
            the m4 tile's second bank (ONE strided exp). est is one
            [128, 5N] tile per head, cols m*N.. matching the eb layout."""
            g, j = h // 4, h % 4
            krow = k8[b][g][32 * j : 32 * (j + 1), :]
            qrow = q8[b][g][32 * j : 32 * (j + 1), :]
            qblk = [
                qrow[:, 0:1024].rearrange("p (i n) -> p i n", i=2),
                qrow[:, 1024:1154].rearrange("p (i n) -> p i n", i=2),
            ]

            def kvm(m):
                return krow[:, 256 * m : 256 * (m + 1)].rearrange(
                    "p (i c) -> p i c", i=2
                )

            # est column layout: [m0..m4 mains 512 each | m0..m4 tails 65
            # each] — ps4's [main | 5 tails] block maps to ONE contiguous
            # [128, 837] exp, and pair exps write contiguous [128, 1024]
            est = estp.tile([128, 5 * N], F16, name=f"est{h}_{b}", tag="est")
            for pi, pr in enumerate(((0, 1), (2, 3))):
                ps = ps_st.tile([128, 1024], F32, name=f"pst{h}_{b}_{pi}", tag="st")
                for idx, m in enumerate(pr):
                    nc.tensor.matmul(
                        ps[:, idx * 512 : (idx + 1) * 512],
                        kvm(m),
                        qblk[0],
                        start=True,
                        stop=True,
                        perf_mode=DR,
                        tile_position=(32 * j, 0),
                    )
                nc.scalar.activation(
                    est[:, pr[0] * 512 : (pr[1] + 1) * 512], ps[:],
                    AF.Exp, scale=EXPSC,
                )
                if fill is not None and inner > pi:
                    fill(1, ev=INNER_EV)
            # m4 main + all five tails, contiguous in psum AND in est
            ps4 = ps_st.tile([128, 1024], F32, name=f"pst{h}_{b}_4", tag="st")
            nc.tensor.matmul(
                ps4[:, 0:512], kvm(4), qblk[0],
                start=True, stop=True, perf_mode=DR, tile_position=(32 * j, 0),
            )
            for m in range(5):
                nc.tensor.matmul(
                    ps4[:, 512 + m * 65 : 512 + (m + 1) * 65],
                    kvm(m),
                    qblk[1],
                    start=True, stop=True, perf_mode=DR, tile_position=(32 * j, 0),
                )
            nc.scalar.activation(
                est[:, 2048 : 2048 + 837], ps4[:, 0:837], AF.Exp, scale=EXPSC
            )
            # bias mults: pair mains (est contiguous vs eb strided), m4 main,
            # then all five tails
            npool = EST_PAIR_POOL_B0 if b == 0 else EST_PAIR_POOL
            for pi, pr in enumerate(((0, 1), (2, 3))):
                v = est[:, pr[0] * 512 : (pr[1] + 1) * 512].rearrange(
                    "p (m n) -> p m n", m=2
                )
                ev = eb[:, pr[0] * N : (pr[1] + 1) * N].rearrange(
                    "p (m n) -> p m n", m=2
                )[:, :, 0:512]
                eng = nc.gpsimd if pi < npool else nc.vector
                eng.tensor_mul(v, v, ev)
            eng4 = nc.gpsimd if EST_M4 == "pool" else nc.vector
            eng4.tensor_mul(
                est[0:65, 2048:2560], est[0:65, 2048:2560],
                eb[0:65, 4 * N : 4 * N + 512],
            )
            tv = est[:, 2560:2885].rearrange("p (m n) -> p m n", m=5)
            te = eb[:, 0 : 5 * N].rearrange("p (m n) -> p m n", m=5)[:, :, 512:N]
            nc.vector.tensor_mul(tv, tv, te)
            return est

        def emit_att_pv(h, b, est):
            ctq = h // 2
            off = (h % 2) * HD

            def pv_mms(pv, n0, nw):
                # lhsT = [ones(64) | ws*v(64)] -> pv rows 0:64 are 64
                # identical copies of the denominator, 64:128 the numerator
                for m, (m0, mw) in enumerate(NT):
                    nc.tensor.matmul(
                        pv[:, n0 : n0 + nw] if PV_MERGE else pv[:, :nw],
                        vt[b][m][:mw, h * 2 * HD : (h + 1) * 2 * HD],
                        est[:mw, m * 512 + n0 : m * 512 + n0 + nw]
                        if n0 == 0
                        else est[:mw, 2560 + m * 65 : 2560 + (m + 1) * 65],
                        start=(m == 0),
                        stop=(m == len(NT) - 1),
                    )

            # ISA: only one non-scalar input may read PSUM, so a fused
            # psum/psum divide is illegal; recip to SBUF then multiply
            if PV_MERGE:
                pv = ps_pv.tile([128, N], F32, name=f"pv{h}_{b}", tag="pv")
                for n0, nw in FB:
                    pv_mms(pv, n0, nw)
                rr = rowp.tile([HD, N], F32, name=f"rr{h}_{b}", tag="rr")
                nc.vector.reciprocal(rr[0:HD, :], pv[0:HD, :])
                nc.vector.tensor_mul(
                    ot[b][ctq][off : off + HD, :],
                    pv[HD : 2 * HD, :],
                    rr[0:HD, :],
                )
            else:
                for fi, (n0, nw) in enumerate(FB):
                    pv = ps_pv.tile([128, 512], F32, name=f"pv{h}_{b}_{fi}", tag="pv")
                    pv_mms(pv, n0, nw)
                    rr = rowp.tile([HD, 512], F32, name=f"rr{h}_{b}_{fi}", tag="rr")
                    nc.vector.reciprocal(rr[0:HD, :nw], pv[0:HD, :nw])
                    nc.vector.tensor_mul(
                        ot[b][ctq][off : off + HD, n0 : n0 + nw],
                        pv[HD : 2 * HD, :nw],
                        rr[0:HD, :nw],
                    )

        def emit_att(h, b, eb, fill=None, inner=0, mid=1):
            est = emit_att_st(h, b, eb, fill=fill, inner=inner)
            if fill is not None and mid:
                fill(mid, ev=INNER_EV)
            emit_att_pv(h, b, est)

        def proj_unit(b, cot, fi, pool=None, tag=None, ev=None):
            """One FB chunk of a proj channel block; bias-add fused into the
            psum eviction (TensorScalarPtr), store per chunk."""
            n0, nw = FB[fi]
            pool = pool or ps_mm
            ps = pool.tile(
                [128, N if tag == "st" else 512], F32,
                name=f"pspu{b}_{cot}_{fi}", tag=tag or "mm",
            )
            for k in range(KC):
                nc.tensor.matmul(
                    ps[:, :nw],
                    pw[:, k * C + cot * 128 : k * C + (cot + 1) * 128],
                    ot[b][k][:, n0 : n0 + nw],
                    start=(k == 0),
                    stop=(k == KC - 1),
                )
            fin = finp.tile([128, N], F16, name=f"finu{b}_{cot}_{fi}", tag="fin")
            if ev == "act":
                nc.scalar.activation(
                    fin[:, n0 : n0 + nw], ps[:, :nw], AF.Identity,
                    bias=pbc[:, cot : cot + 1],
                )
            else:
                nc.vector.tensor_scalar_add(
                    fin[:, n0 : n0 + nw], ps[:, :nw], pbc[:, cot : cot + 1]
                )
            nc.sync.dma_start(
                out_d[b, cot * 128 : (cot + 1) * 128, n0 : n0 + nw],
                fin[:, n0 : n0 + nw],
            )

        def proj_unit_split(b, cot, fi):
            """proj chunk as two half-contraction fill units (~640ns each) so
            a fill never stalls the in-order PE stream for a full chunk."""
            st = {}
            n0, nw = FB[fi]

            def mm(ps, k):
                nc.tensor.matmul(
                    ps[:, :nw],
                    pw[:, k * C + cot * 128 : k * C + (cot + 1) * 128],
                    ot[b][k][:, n0 : n0 + nw],
                    start=(k == 0),
                    stop=(k == KC - 1),
                )

            def ua(ev=None):
                st["ps"] = ps_mm.tile([128, 512], F32, name=f"psps{b}_{cot}_{fi}", tag="mm")
                for k in range(3):
                    mm(st["ps"], k)

            def ub(ev=None):
                ps = st["ps"]
                for k in range(3, KC):
                    mm(ps, k)
                fin = finp.tile([128, N], F16, name=f"fins{b}_{cot}_{fi}", tag="fin")
                nc.vector.tensor_scalar_add(
                    fin[:, n0 : n0 + nw], ps[:, :nw], pbc[:, cot : cot + 1]
                )
                nc.sync.dma_start(
                    out_d[b, cot * 128 : (cot + 1) * 128, n0 : n0 + nw],
                    fin[:, n0 : n0 + nw],
                )
            return [ua, ub]

        def emit_proj(b, cot, tail=False):
            """In the tail (attention done) rotate over all three psum pools
            so the eviction latency never gates the next group; DVE and ACT
            are both idle there, so alternate eviction engines, merge the two
            chunk evictions into one fin tile (one store per cot), and rotate
            the store queue so the final drain isn't serial on SP."""
            if not tail:
                for fi in range(2):
                    proj_unit(b, cot, fi)
                return
            fin = finp.tile([128, N], F16, name=f"fint{b}_{cot}", tag="fin")
            for fi, (n0, nw) in enumerate(FB):
                r = (2 * cot + fi) % 3
                pool, tag = [(ps_mm, "mm"), (ps_pv, "pv"), (ps_st, "st")][r]
                ps = pool.tile(
                    [128, N if tag == "st" else 512], F32,
                    name=f"pspt{b}_{cot}_{fi}", tag=tag,
                )
                for k in range(KC):
                    nc.tensor.matmul(
                        ps[:, :nw],
                        pw[:, k * C + cot * 128 : k * C + (cot + 1) * 128],
                        ot[b][k][:, n0 : n0 + nw],
                        start=(k == 0),
                        stop=(k == KC - 1),
                    )
                if (2 * cot + fi) % 2 == 0:
                    nc.scalar.activation(
                        fin[:, n0 : n0 + nw], ps[:, :nw], AF.Identity,
                        bias=pbc[:, cot : cot + 1],
                    )
                else:
                    nc.vector.tensor_scalar_add(
                        fin[:, n0 : n0 + nw], ps[:, :nw], pbc[:, cot : cot + 1]
                    )
            eng = [nc.sync, nc.gpsimd, nc.scalar][cot % 3]
            eng.dma_start(out_d[b, cot * 128 : (cot + 1) * 128, :], fin[:, :])

        # ---- emission schedule: fill PE during ACT/DVE-bound attention ----
        # merged pass 1: b0 QKV feeds b0 attention head-pairs immediately so
        # the exp/mult/norm stream starts early; b1 QKV interleaved.
        # PE is in-order, so any stalled instruction blocks everything behind
        # it: feed independent fill units (qkv/proj chunks) between attention
        # pieces from a queue, ordered by their consume-by deadlines
        def mkfill(units):
            q = deque(units)

            def fill(n, ev=None):
                for _ in range(min(n, len(q))):
                    q.popleft()(ev)
                return len(q)
            fill.q = q
            return fill

        # ci0 v first (tracks the SP stream), then cluster-0 q/k (tracks the
        # ACT-queue wqk/x8q stream) so head 0's exp chain starts ASAP; the
        # ci1 v units slot in as the first head's fills
        emit_qkv_v0_startup()
        emit_qkv_qk0()

        def qk_units(b, ts, force_ev=None):
            return [
                lambda ev=None, t=t, fc=fc: qk_unit(b, t, fc, ev=force_ev or ev)
                for t in ts
                for fc in range(2)
            ]

        u1 = []
        u1 += qk_units(0, range(4, 8))
        u1 += [lambda ev=None, m=m: emit_qkv_v(0, m, cis=(1,), ev=ev) for m in range(5)]
        u1 += [lambda ev=None, m=m: emit_qkv_v(1, m, cis=(0,), ev=ev) for m in range(3)]
        u1 += qk_units(0, range(8, 12))
        u1 += [lambda ev=None, m=m: emit_qkv_v(1, m, cis=(0,), ev=ev) for m in (3, 4)]
        u1 += qk_units(1, range(0, 4))
        fill1 = mkfill(u1)
        for h in range(H):
            per = -(-len(fill1.q) // (H - h))  # ceil: spread fills over heads
            fill1(min(2, max(1, per - 2)))
            emit_att(h, 0, ebt[h], fill=fill1, inner=min(2, max(0, per - 2)), mid=1 if per >= 2 else 0)
            if h == 5:
                # proj weights aren't consumed until pass 2; issue them here
                # so they never queue ahead of the eb bias tiles on SP
                nc.sync.dma_start(
                    pw[:].rearrange("p (k c) -> p k c", k=KC),
                    pw_d[:].rearrange("(k p) c -> p k c", p=128),
                )
                nc.sync.dma_start(pbc[:], pbc_d[:])
        while fill1(1):
            pass
        # pass 2: b1 attention; fillers: remaining b1 qk groups early (group
        # 4+h consumed at head 4+(h%4) of the next cluster), b0 proj late
        u2 = []
        u2 += [lambda ev=None, m=m: emit_qkv_v(1, m, cis=(1,), ev=ev) for m in range(5)]
        u2 += qk_units(1, range(4, 12))
        u2 += [lambda ev=None, cot=cot, fi=fi: proj_unit(0, cot, fi) for cot in range(5) for fi in (0, 1)]
        fill2 = mkfill(u2)
        for h in range(H):
            per = -(-len(fill2.q) // (H - h))
            cap = 3 if h <= 6 else 2
            fill2(min(cap, max(1, per - 1)))
            if h == H - 1:
                est_last = emit_att_st(h, 1, ebt[h], fill=fill2, inner=0)
                proj_unit(0, 5, 0)
                proj_unit(0, 5, 1)
                emit_att_pv(h, 1, est_last)
            else:
                emit_att(h, 1, ebt[h], fill=fill2, inner=min(2, max(0, per - 2)), mid=1 if per >= 2 else 0)
        while fill2(1):
            pass
        # proj(0,5) here fills the last head's recip/mult latency
        # that gates proj(1,·)'s final accumulation steps
        for cot in range(KC):
            emit_proj(1, cot, tail=True)

    nc.compile()
    return nc


_NC = None


def get_compiled():
    global _NC
    if _NC is None:
        _NC = build_graph()
    return _NC


def prep_in_maps(x, rel_pos_bias, qkv_weight, q_bias, v_bias, proj_weight, proj_bias):
    x = np.asarray(x, np.float32)
    rel_pos_bias = np.asarray(rel_pos_bias, np.float32)
    qkv_weight = np.asarray(qkv_weight, np.float32)
    q_bias = np.asarray(q_bias, np.float32)
    v_bias = np.asarray(v_bias, np.float32)
    proj_weight = np.asarray(proj_weight, np.float32)
    proj_bias = np.asarray(proj_bias, np.float32)

    xT = np.ascontiguousarray(x.transpose(0, 2, 1))  # (B, C, N) fp32
    x8 = xT.astype(NPF8)
    xr = (xT - x8.astype(np.float32)).astype(NPF8)

    def pack_x(a):
        """(B, C, N) fp8 -> q-layout [B,128,KB*2N] and m-layout [B,128,KB*1280].

        R[b, kb, i, p, n] holds channel c = kb*256 + i*128 + p. Every DR
        block is stored (i, cols)-contiguous per partition; m4 zero-padded.
        """
        R = a.reshape(B, KB, 2, 128, N)
        to2d = lambda t: np.ascontiguousarray(
            t.transpose(0, 2, 1, 3).reshape(t.shape[0], 128, -1))
        xq = np.concatenate(
            [R[:, :, 0, :, 0:512], R[:, :, 1, :, 0:512],
             R[:, :, 0, :, 512:N], R[:, :, 1, :, 512:N]], axis=-1
        )  # (B, KB, 128, 2N) with per-kb cols [i0 512 | i1 512 | i0 65 | i1 65]
        pad = np.zeros(R.shape[:1] + (KB, 128, 128 - 65), a.dtype)
        mparts = []
        for m0, mw in NT:
            for i in range(2):
                mparts.append(R[:, :, i, :, m0:m0 + mw])
                if mw < 128:
                    mparts.append(pad)
        xm = np.concatenate(mparts, axis=-1)
        return to2d(xq), to2d(xm)

    x8q, x8m = pack_x(x8)
    _, xrm = pack_x(xr)

    # permute wqk output columns into the fp8-DoubleRow eviction layout:
    # group t = 4g+part covers heads 4g+j at rows 32j+c; part 0/1 = q channel
    # halves, part 2/3 = k halves. Weights carry WS (e4m3 subnormal guard);
    # the per-part eviction scale QEVS/KEVS restores q*SCALE*QS8 / k*KS8.
    wqk_perm = np.empty((2 * C, C), np.float32)
    qkb = np.zeros((128, 2 * KC), np.float32)
    for g in range(3):
        for part in range(4):
            t = 4 * g + part
            i = part % 2
            for j in range(4):
                h = 4 * g + j
                rows = h * HD + i * 32 + np.arange(32)
                cols = t * 128 + 32 * j + np.arange(32)
                if part < 2:
                    wqk_perm[cols] = qkv_weight[rows] * WS
                    qkb[32 * j : 32 * (j + 1), t] = q_bias[rows] * (SCALE * QS8)
                else:
                    wqk_perm[cols] = qkv_weight[C + rows] * WS
    # stationary layout [128, (kb, t, i, 128)]
    WQ = wqk_perm.T.astype(NPF8).reshape(KB, 2, 128, 2 * C)  # (kb, i, p, out)
    wqkT8 = np.ascontiguousarray(
        WQ.reshape(KB, 2, 128, 12, 128).transpose(2, 0, 3, 1, 4)
        .reshape(128, KB * 12 * 256)
    )

    wvT = np.ascontiguousarray(qkv_weight[2 * C :].T) * WS  # (C, C) fp32
    wv8f = wvT.astype(NPF8)
    wvr8f = (wvT - wv8f.astype(np.float32)).astype(NPF8)

    def pack_wv(a):
        # moving layout [128, (kb, ci, i, 384)]
        R = a.reshape(KB, 2, 128, 2, CC[0][1])  # (kb, i, p, ci, 384)
        return np.ascontiguousarray(
            R.transpose(2, 0, 3, 1, 4).reshape(128, KB * 2 * C))

    wv8 = pack_wv(wv8f)
    wvr8 = pack_wv(wvr8f)

    # vt carries WS*v; fold 1/WS into the fp16 proj weights
    pwT = np.ascontiguousarray(proj_weight.T / WS).astype(NPF16)  # (C, C)
    ebT = np.zeros((H, 640, N), NPF16)
    ebT[:, :N, :] = np.exp(
        rel_pos_bias.transpose(0, 2, 1).astype(np.float64)
    ).astype(NPF16)
    ebT = np.ascontiguousarray(ebT).reshape(H * 640, N)

    pbe = (proj_bias + v_bias @ proj_weight.T).astype(np.float32)  # (C,)
    pbc = np.ascontiguousarray(pbe.reshape(KC, 128).T)  # [p, cot] = pbe[cot*128+p]

    shared = {
        "wqk8": wqkT8, "wv8": wv8, "wvr8": wvr8, "pwT": pwT, "ebT": ebT,
        "qkb": qkb, "pbc": pbc,
    }
    in_maps = []
    for i in range(NCORES):
        m = dict(shared)
        m["x8q"] = np.ascontiguousarray(x8q[i * BL : (i + 1) * BL])
        m["x8m"] = np.ascontiguousarray(x8m[i * BL : (i + 1) * BL])
        m["xrm"] = np.ascontiguousarray(xrm[i * BL : (i + 1) * BL])
        in_maps.append(m)
    return in_maps


def run(inputs, trace=False, **kw):
    nc = get_compiled()
    in_maps = prep_in_maps(**inputs)
    res = run_bass_kernel_spmd(nc, in_maps, core_ids=list(range(NCORES)), trace=trace, **kw)
    outT = np.concatenate(
        [np.asarray(r["out"], dtype=np.float32) for r in res.results], axis=0
    )  # (B, C, N)
    out = np.ascontiguousarray(outT.transpose(0, 2, 1))
    return out, res


def kernel(**inputs):
    out, _ = run(inputs, trace=False)
    return out
